# revision 18
# baseline (speedup 1.0000x reference)
"""SSD-style CustomLoss (Huber loc loss + hard-negative-mined CE conf loss)
as a Trainium2 Bass/Tile kernel, data-parallel over the batch axis on 8
NeuronCores.

Per-core device work (8 images each):
  - CE-from-logits (logsumexp - <y, x>) per box, used to rank negatives
  - CE-from-probs (normalize / clip / log) per box
  - Huber loc loss on positive boxes
  - per-image top-k negative selection via on-device threshold bisection
  - masked sums -> 3 scalar partials per core
Host: pad/reshape inputs, gather the per-core scalar partials, all-reduce
total_pos, final division.
"""

import os

import numpy as np

import concourse.bass as bass
import concourse.mybir as mybir
from concourse.bass_utils import run_bass_kernel_spmd
from concourse.mybir import ActivationFunctionType as Act
from concourse.mybir import AluOpType as Op
from concourse.tile import TileContext, add_dep_helper

B, N, C = 64, 8732, 21
NCORES = 8
NIMG = B // NCORES  # images per core
F = 69  # tokens per partition (padded): 128 * 69 = 8832 >= 8732
NPAD = 128 * F
NEG_POS_RATIO = 3.0
EPS = 1e-7
BIG_NEG = -1.0e30
T_BISECT = 28
F32 = mybir.dt.float32
X = mybir.AxisListType.X
XY = mybir.AxisListType.XY

# Results of the last device run (exec time etc), for the test harness.
LAST_RESULTS = None

# The walrus build in this container rejects instructions carrying more than
# MAX_WAITS semaphore waits ("Too many sync wait commands"). Tile's scheduler
# freely emits 3+ waits per instruction, so split the excess onto NoOps
# inserted just before the offending instruction (same engine => executes
# before it in the engine's program order).
MAX_WAITS = 1       # per compute/DMA instruction
NOP_WAITS = 1       # per inserted NoOp (same 1-wait limit)


def _split_excess_waits(bir_json: bytes) -> bytes:
    import json as _json

    m = _json.loads(bir_json)
    ctr = 0
    for fdef in m["functions"]:
        for blk in fdef["blocks"]:
            insts = blk["instructions"]
            out = []
            for ins in insts:
                si = ins.get("sync_info")
                ow = (si or {}).get("on_wait") or []
                cap = NOP_WAITS if ins.get("opcode") in ("NoOp", "Drain") else MAX_WAITS
                if len(ow) > cap:
                    keep = ow[-cap:]
                    excess = ow[:-cap]
                    si["on_wait"] = keep
                    while excess:
                        chunk, excess = excess[:NOP_WAITS], excess[NOP_WAITS:]
                        ctr += 1
                        out.append(
                            {
                                "debug": ins.get("debug"),
                                "engine": ins["engine"],
                                "ins": [],
                                "name": f"I-wsplit-{ctr}",
                                "opcode": "NoOp",
                                "outs": [],
                                "sync_info": {"on_update": [], "on_wait": chunk},
                            }
                        )
                out.append(ins)
            blk["instructions"] = out
    return _json.dumps(m).encode()


def _patch_wait_splitting(nc):
    orig = nc.to_json_bytes

    def patched():
        return _split_excess_waits(orig())

    nc.to_json_bytes = patched
    return nc


def emit_program(nc, pl, al, pd, ad, msk, out, n_img, f):
    """Emit the per-core program. pl/al: [n_img, 128*f, C]; pd/ad:
    [n_img, 128*f, 4]; msk: [128, f] (1 = real token); out: [1, 4] =
    (sum hub4*pos, sum <y,log p>*sel, total_pos, unused)."""
    fc = f * C
    f4 = f * 4

    from contextlib import ExitStack

    with TileContext(nc) as tc, ExitStack() as stk:
        per = stk.enter_context(tc.tile_pool(name="per", bufs=1))
        ip = stk.enter_context(tc.tile_pool(name="img", bufs=2))
        pp = stk.enter_context(tc.tile_pool(name="ps", bufs=2, space="PSUM"))

        mskt = per.tile([128, f], F32)
        nc.sync.dma_start(mskt[:], msk[:])

        # persistent per-core maps
        mrm = per.tile([128, n_img * f], F32)   # masked ranking values
        cp = per.tile([128, n_img * f], F32)    # <y, log p> per box
        posm = per.tile([128, n_img * f], F32)  # positive mask
        hpp = per.tile([128, n_img * f], F32)   # hub4 * pos
        pc_img = per.tile([128, n_img], F32)    # per-partition pos counts
        ones128 = per.tile([128, 128], F32)
        nc.vector.memset(ones128[:], 1.0)
        nc.gpsimd.memset(mrm[:], BIG_NEG)

        for b in range(n_img):
            xt = ip.tile([128, fc], F32, tag="xt")
            at = ip.tile([128, fc], F32, tag="at")
            pdt = ip.tile([128, f4], F32, tag="pdt")
            adt = ip.tile([128, f4], F32, tag="adt")
            nc.sync.dma_start(xt[:], pl[b].rearrange("(p f) c -> p (f c)", p=128))
            nc.sync.dma_start(at[:], al[b].rearrange("(p f) c -> p (f c)", p=128))
            nc.sync.dma_start(pdt[:], pd[b].rearrange("(p f) c -> p (f c)", p=128))
            nc.sync.dma_start(adt[:], ad[b].rearrange("(p f) c -> p (f c)", p=128))

            x3 = xt[:].rearrange("p (f c) -> p f c", c=C)
            a3 = at[:].rearrange("p (f c) -> p f c", c=C)
            bf = slice(b * f, (b + 1) * f)

            # --- CE from logits: mr = log(sum exp x) - <y, x> ---
            e = ip.tile([128, fc], F32, tag="e")
            nc.scalar.activation(e[:], xt[:], Act.Exp)
            s1 = ip.tile([128, f], F32, tag="s1")
            nc.vector.reduce_sum(s1[:], e[:].rearrange("p (f c) -> p f c", c=C), axis=X)
            axp = ip.tile([128, fc], F32, tag="axp")
            nc.gpsimd.tensor_mul(axp[:], at[:], xt[:])
            ax = ip.tile([128, f], F32, tag="ax")
            nc.vector.reduce_sum(ax[:], axp[:].rearrange("p (f c) -> p f c", c=C), axis=X)
            lse = ip.tile([128, f], F32, tag="lse")
            nc.scalar.activation(lse[:], s1[:], Act.Ln)
            mr = ip.tile([128, f], F32, tag="mr")
            nc.vector.tensor_sub(mr[:], lse[:], ax[:])

            # --- CE from probs: cp = <y, log clip(x / sum x)> ---
            s2 = ip.tile([128, f], F32, tag="s2")
            nc.vector.reduce_sum(s2[:], x3, axis=X)
            r2 = ip.tile([128, f], F32, tag="r2")
            nc.vector.reciprocal(r2[:], s2[:])
            p = ip.tile([128, fc], F32, tag="p")
            r2b = r2[:, :, None].broadcast_to([128, f, C])
            nc.vector.tensor_mul(p[:].rearrange("p (f c) -> p f c", c=C), x3, r2b)
            nc.gpsimd.tensor_scalar(p[:], p[:], EPS, 1.0 - EPS, Op.max, Op.min)
            lp = ip.tile([128, fc], F32, tag="lp")
            nc.scalar.activation(lp[:], p[:], Act.Ln)
            alpp = ip.tile([128, fc], F32, tag="alpp")
            nc.gpsimd.tensor_mul(alpp[:], at[:], lp[:])
            nc.vector.reduce_sum(
                cp[:, bf], alpp[:].rearrange("p (f c) -> p f c", c=C), axis=X
            )

            # --- Huber (sum over the 4 coords; /4 folded into host) ---
            d3v = lambda t: t[:].rearrange("p (f c) -> p f c", c=4)
            dd = ip.tile([128, f4], F32, tag="dd")
            nc.vector.tensor_sub(dd[:], pdt[:], adt[:])
            absd = ip.tile([128, f4], F32, tag="absd")
            nc.scalar.activation(absd[:], dd[:], Act.Abs)
            m = ip.tile([128, f4], F32, tag="m")
            nc.vector.tensor_scalar_min(m[:], absd[:], 1.0)
            sq = ip.tile([128, f4], F32, tag="sq")
            nc.gpsimd.tensor_mul(sq[:], m[:], m[:])
            rad = ip.tile([128, f], F32, tag="rad")
            nc.vector.reduce_sum(rad[:], d3v(absd), axis=X)
            rm = ip.tile([128, f], F32, tag="rm")
            nc.vector.reduce_sum(rm[:], d3v(m), axis=X)
            rsq = ip.tile([128, f], F32, tag="rsq")
            nc.vector.reduce_sum(rsq[:], d3v(sq), axis=X)
            # hub4 = 0.5*rsq + rad - rm
            h1 = ip.tile([128, f], F32, tag="h1")
            nc.vector.scalar_tensor_tensor(h1[:], rsq[:], 0.5, rad[:], Op.mult, Op.add)
            hub = ip.tile([128, f], F32, tag="hub")
            nc.vector.scalar_tensor_tensor(hub[:], rm[:], -1.0, h1[:], Op.mult, Op.add)
            # positives: any |actual delta| > 0
            absa = ip.tile([128, f4], F32, tag="absa")
            nc.scalar.activation(absa[:], adt[:], Act.Abs)
            pm = ip.tile([128, f], F32, tag="pm")
            nc.vector.tensor_reduce(pm[:], d3v(absa), axis=X, op=Op.max)
            nc.vector.tensor_scalar(posm[:, bf], pm[:], 0.0, None, Op.is_gt)
            nc.vector.tensor_mul(hpp[:, bf], hub[:], posm[:, bf])

            # --- ranking mask: valid negatives only ---
            nv = ip.tile([128, f], mybir.dt.int32, tag="nv")
            nc.vector.tensor_sub(nv[:], mskt[:], posm[:, bf])
            nc.vector.copy_predicated(mrm[:, bf], nv[:], mr[:])

            # per-image pos counts (per-partition partials)
            nc.vector.reduce_sum(pc_img[:, b : b + 1], posm[:, bf], axis=X)

        # ---- cross-partition totals ----
        kps = pp.tile([128, n_img], F32)
        nc.tensor.matmul(kps[:], ones128[:], pc_img[:], start=True, stop=True)
        kimg = per.tile([128, n_img], F32)
        nc.vector.tensor_scalar(kimg[:], kps[:], NEG_POS_RATIO, None, Op.mult)

        # fixed bisection bounds (no cross-partition min/max primitive here);
        # |mr| is bounded by ~|logsumexp| + C*max|y*x| << 1e4 for sane inputs.
        hi_t = per.tile([128, n_img], F32)
        nc.vector.memset(hi_t[:], 1.0e4)
        lo_t = per.tile([128, n_img], F32)
        nc.vector.memset(lo_t[:], -1.0e4)

        # ---- bisection for per-image rank-k threshold ----
        mr3 = mrm[:].rearrange("p (b f) -> p b f", b=n_img)
        mid = per.tile([128, n_img], F32)
        cmp_t = per.tile([128, n_img * f], F32)
        cmp3 = cmp_t[:].rearrange("p (b f) -> p b f", b=n_img)
        cnt = per.tile([128, n_img], F32)
        junk = per.tile([128, 1], F32)
        ge = per.tile([128, n_img], mybir.dt.int32)
        lt = per.tile([128, n_img], mybir.dt.int32)
        for _t in range(T_BISECT):
            nc.vector.tensor_add(mid[:], lo_t[:], hi_t[:])
            nc.vector.tensor_scalar_mul(mid[:], mid[:], 0.5)
            midb = mid[:, :, None].broadcast_to([128, n_img, f])
            nc.vector.tensor_tensor(cmp3, mr3, midb, op=Op.is_ge)
            nc.vector.reduce_sum(cnt[:], cmp3, axis=X)
            cps = pp.tile([128, n_img], F32, tag="cps")
            nc.tensor.matmul(cps[:], ones128[:], cnt[:], start=True, stop=True)
            nc.vector.tensor_tensor(ge[:], cps[:], kimg[:], op=Op.is_ge)
            nc.vector.tensor_tensor(lt[:], cps[:], kimg[:], op=Op.is_lt)
            nc.vector.copy_predicated(lo_t[:], ge[:], mid[:])
            nc.vector.copy_predicated(hi_t[:], lt[:], mid[:])

        # ---- final masked sums ----
        lob = lo_t[:, :, None].broadcast_to([128, n_img, f])
        nc.vector.tensor_tensor(cmp3, mr3, lob, op=Op.is_ge)  # selected negs
        nc.vector.tensor_add(cmp_t[:], cmp_t[:], posm[:])     # | positives
        sc = per.tile([128, n_img * f], F32)
        csum = per.tile([128, 1], F32)
        nc.vector.tensor_mul(sc[:], cp[:], cmp_t[:])
        nc.vector.reduce_sum(csum[:], sc[:], axis=X)
        hsum = per.tile([128, 1], F32)
        nc.vector.reduce_sum(hsum[:], hpp[:], axis=X)
        ptot = per.tile([128, 1], F32)
        nc.vector.reduce_sum(ptot[:], pc_img[:], axis=X)

        pk = per.tile([128, 4], F32)
        nc.vector.memset(pk[:], 0.0)
        nc.vector.tensor_copy(pk[:, 0:1], hsum[:])
        nc.vector.tensor_copy(pk[:, 1:2], csum[:])
        nc.vector.tensor_copy(pk[:, 2:3], ptot[:])
        pkr = pp.tile([128, 4], F32)
        nc.tensor.matmul(pkr[:], ones128[:], pk[:], start=True, stop=True)
        outt = per.tile([1, 4], F32)
        i_cp = nc.vector.tensor_copy(outt[:], pkr[0:1, :])
        i_dma = nc.sync.dma_start(out[:], outt[:])

        # funnel waits so the tail drain needs few sem waits
        n1 = nc.sync.nop()
        add_dep_helper(n1.ins, i_cp.ins, sync=True, reason="funnel-dve")
        n2 = nc.sync.nop()
        add_dep_helper(n2.ins, i_dma.ins, sync=True, reason="funnel-dma")

    return nc


def build_bass(n_img=NIMG, f=F):
    np_tok = 128 * f
    nc = bass.Bass()
    pl = nc.dram_tensor("pl", [n_img, np_tok, C], F32, kind="ExternalInput")
    al = nc.dram_tensor("al", [n_img, np_tok, C], F32, kind="ExternalInput")
    pd = nc.dram_tensor("pd", [n_img, np_tok, 4], F32, kind="ExternalInput")
    ad = nc.dram_tensor("ad", [n_img, np_tok, 4], F32, kind="ExternalInput")
    msk = nc.dram_tensor("msk", [128, f], F32, kind="ExternalInput")
    out = nc.dram_tensor("out", [1, 4], F32, kind="ExternalOutput")
    emit_program(nc, pl, al, pd, ad, msk, out, n_img, f)
    return _patch_wait_splitting(nc)


def _pad_tokens(x, npad, fill):
    """[B, N, D] -> [B, npad, D] padded with `fill` along tokens."""
    b, n, d = x.shape
    if n == npad:
        return np.ascontiguousarray(x, dtype=np.float32)
    out = np.full((b, npad, d), fill, dtype=np.float32)
    out[:, :n, :] = x
    return out


def kernel(actual_bbox_deltas, actual_labels, pred_bbox_deltas, pred_labels):
    global LAST_RESULTS
    ab = np.asarray(actual_bbox_deltas, dtype=np.float32)
    al_ = np.asarray(actual_labels, dtype=np.float32)
    pb = np.asarray(pred_bbox_deltas, dtype=np.float32)
    pl_ = np.asarray(pred_labels, dtype=np.float32)
    assert pl_.shape == (B, N, C), pl_.shape

    # Pad tokens to 128*F. Padded pred_labels rows are all-ones (safe for
    # exp/log); padded labels/deltas are zero, and the msk input excludes
    # padded tokens from negative mining.
    plp = _pad_tokens(pl_, NPAD, 1.0)
    alp = _pad_tokens(al_, NPAD, 0.0)
    pbp = _pad_tokens(pb, NPAD, 0.0)
    abp = _pad_tokens(ab, NPAD, 0.0)

    tok = np.arange(NPAD).reshape(128, F)
    msk = (tok < N).astype(np.float32)

    nc = build_bass()
    in_maps = []
    for c in range(NCORES):
        sl = slice(c * NIMG, (c + 1) * NIMG)
        in_maps.append(
            {
                "pl": np.ascontiguousarray(plp[sl]),
                "al": np.ascontiguousarray(alp[sl]),
                "pd": np.ascontiguousarray(pbp[sl]),
                "ad": np.ascontiguousarray(abp[sl]),
                "msk": msk,
            }
        )

    trace = bool(int(os.environ.get("KERNEL_TRACE", "0")))
    res = run_bass_kernel_spmd(
        nc, in_maps, core_ids=list(range(NCORES)), trace=trace
    )
    LAST_RESULTS = res

    hub_sum = 0.0
    cesel_sum = 0.0
    pos_total = 0.0
    for r in res.results:
        o = r["out"].reshape(-1)
        hub_sum += float(o[0])
        cesel_sum += float(o[1])
        pos_total += float(o[2])

    total_pos = max(pos_total, 1.0)
    loc_loss = np.float32(0.25 * hub_sum / total_pos)
    conf_loss = np.float32(-cesel_sum / total_pos)
    return loc_loss, conf_loss


# revision 24
# speedup vs baseline: 1.7950x; 1.7950x over previous
"""SSD-style CustomLoss (Huber loc loss + hard-negative-mined CE conf loss)
as a Trainium2 Bass/Tile kernel, data-parallel over the batch axis on 8
NeuronCores.

Per-core device work (8 images each):
  - CE-from-logits (logsumexp - <y, x>) per box, used to rank negatives
  - CE-from-probs (normalize / clip / log) per box
  - Huber loc loss on positive boxes
  - per-image top-k negative selection via on-device threshold bisection
  - masked sums -> 3 scalar partials per core
Host: pad/reshape inputs, gather the per-core scalar partials, all-reduce
total_pos, final division.
"""

import os

import numpy as np

import concourse.bass as bass
import concourse.mybir as mybir
from concourse.bass_utils import run_bass_kernel_spmd
from concourse.mybir import ActivationFunctionType as Act
from concourse.mybir import AluOpType as Op
from concourse.tile import TileContext, add_dep_helper

B, N, C = 64, 8732, 21
NCORES = 8
NIMG = B // NCORES  # images per core
F = 69  # tokens per partition (padded): 128 * 69 = 8832 >= 8732
NPAD = 128 * F
NEG_POS_RATIO = 3.0
EPS = 1e-7
BIG_NEG = -1.0e30
T_BISECT = 20
BISECT_BOUND = 200.0  # |mr| is bounded by ~max|lse| + C*max|y*x| << 200 here
F32 = mybir.dt.float32
X = mybir.AxisListType.X
XY = mybir.AxisListType.XY

# Results of the last device run (exec time etc), for the test harness.
LAST_RESULTS = None

# The walrus build in this container rejects instructions carrying more than
# MAX_WAITS semaphore waits ("Too many sync wait commands"). Tile's scheduler
# freely emits 3+ waits per instruction, so split the excess onto NoOps
# inserted just before the offending instruction (same engine => executes
# before it in the engine's program order).
MAX_WAITS = 1       # per compute/DMA instruction
NOP_WAITS = 1       # per inserted NoOp (same 1-wait limit)


def _split_excess_waits(bir_json: bytes) -> bytes:
    import json as _json

    m = _json.loads(bir_json)
    ctr = 0
    for fdef in m["functions"]:
        for blk in fdef["blocks"]:
            insts = blk["instructions"]
            out = []
            for ins in insts:
                si = ins.get("sync_info")
                ow = (si or {}).get("on_wait") or []
                cap = NOP_WAITS if ins.get("opcode") in ("NoOp", "Drain") else MAX_WAITS
                if len(ow) > cap:
                    keep = ow[-cap:]
                    excess = ow[:-cap]
                    si["on_wait"] = keep
                    while excess:
                        chunk, excess = excess[:NOP_WAITS], excess[NOP_WAITS:]
                        ctr += 1
                        out.append(
                            {
                                "debug": ins.get("debug"),
                                "engine": ins["engine"],
                                "ins": [],
                                "name": f"I-wsplit-{ctr}",
                                "opcode": "NoOp",
                                "outs": [],
                                "sync_info": {"on_update": [], "on_wait": chunk},
                            }
                        )
                out.append(ins)
            blk["instructions"] = out
    return _json.dumps(m).encode()


def _patch_wait_splitting(nc):
    orig = nc.to_json_bytes

    def patched():
        return _split_excess_waits(orig())

    nc.to_json_bytes = patched
    return nc


def emit_program(nc, pl, al, pd, ad, msk, out, n_img, f):
    """Emit the per-core program. pl/al: [n_img, 128*f, C]; pd/ad:
    [n_img, 128*f, 4]; msk: [128, f] (1 = real token); out: [1, 4] =
    (sum hub4*pos, sum <y,log p>*sel, total_pos, unused)."""
    fc = f * C
    f4 = f * 4

    from contextlib import ExitStack

    with TileContext(nc) as tc, ExitStack() as stk:
        per = stk.enter_context(tc.tile_pool(name="per", bufs=1))
        ip = stk.enter_context(tc.tile_pool(name="img", bufs=3))
        pp = stk.enter_context(tc.tile_pool(name="ps", bufs=2, space="PSUM"))

        mskt = per.tile([128, f], F32)
        nc.sync.dma_start(mskt[:], msk[:])

        # persistent per-core maps
        mrm = per.tile([128, n_img * f], F32)   # masked ranking values
        cp = per.tile([128, n_img * f], F32)    # <y, log p> per box
        posm = per.tile([128, n_img * f], F32)  # positive mask
        hpp = per.tile([128, n_img * f], F32)   # hub4 * pos
        pc_img = per.tile([128, n_img], F32)    # per-partition pos counts
        ones128 = per.tile([128, 128], F32)
        nc.vector.memset(ones128[:], 1.0)
        nc.gpsimd.memset(mrm[:], BIG_NEG)

        for b in range(n_img):
            xt = ip.tile([128, fc], F32, tag="xt")
            at = ip.tile([128, fc], F32, tag="at")
            pdt = ip.tile([128, f4], F32, tag="pdt")
            adt = ip.tile([128, f4], F32, tag="adt")
            nc.sync.dma_start(xt[:], pl[b].rearrange("(p f) c -> p (f c)", p=128))
            nc.sync.dma_start(at[:], al[b].rearrange("(p f) c -> p (f c)", p=128))
            nc.sync.dma_start(pdt[:], pd[b].rearrange("(p f) c -> p (f c)", p=128))
            nc.sync.dma_start(adt[:], ad[b].rearrange("(p f) c -> p (f c)", p=128))

            x3 = xt[:].rearrange("p (f c) -> p f c", c=C)
            a3 = at[:].rearrange("p (f c) -> p f c", c=C)
            bf = slice(b * f, (b + 1) * f)

            # --- Huber (sum over the 4 coords; /4 folded into host) ---
            d3v = lambda t: t[:].rearrange("p (f c) -> p f c", c=4)
            dd = ip.tile([128, f4], F32, tag="dd")
            nc.vector.tensor_sub(dd[:], pdt[:], adt[:])
            absd = ip.tile([128, f4], F32, tag="absd")
            nc.scalar.activation(absd[:], dd[:], Act.Abs)
            m = ip.tile([128, f4], F32, tag="m")
            nc.vector.tensor_scalar_min(m[:], absd[:], 1.0)
            sq = ip.tile([128, f4], F32, tag="sq")
            nc.gpsimd.tensor_mul(sq[:], m[:], m[:])
            rad = ip.tile([128, f], F32, tag="rad")
            nc.vector.reduce_sum(rad[:], d3v(absd), axis=X)
            rm = ip.tile([128, f], F32, tag="rm")
            nc.vector.reduce_sum(rm[:], d3v(m), axis=X)
            rsq = ip.tile([128, f], F32, tag="rsq")
            nc.vector.reduce_sum(rsq[:], d3v(sq), axis=X)
            # hub4 = 0.5*rsq + rad - rm
            h1 = ip.tile([128, f], F32, tag="h1")
            nc.vector.scalar_tensor_tensor(h1[:], rsq[:], 0.5, rad[:], Op.mult, Op.add)
            hub = ip.tile([128, f], F32, tag="hub")
            nc.vector.scalar_tensor_tensor(hub[:], rm[:], -1.0, h1[:], Op.mult, Op.add)
            # positives: any |actual delta| > 0
            absa = ip.tile([128, f4], F32, tag="absa")
            nc.scalar.activation(absa[:], adt[:], Act.Abs)
            pm = ip.tile([128, f], F32, tag="pm")
            nc.vector.tensor_reduce(pm[:], d3v(absa), axis=X, op=Op.max)
            nc.vector.tensor_scalar(posm[:, bf], pm[:], 0.0, None, Op.is_gt)
            nc.vector.tensor_mul(hpp[:, bf], hub[:], posm[:, bf])
            nc.vector.reduce_sum(pc_img[:, b : b + 1], posm[:, bf], axis=X)

            # --- CE from logits: mr = log(sum exp x) - <y, x> ---
            e = ip.tile([128, fc], F32, tag="e")
            nc.scalar.activation(e[:], xt[:], Act.Exp)
            s1 = ip.tile([128, f], F32, tag="s1")
            nc.vector.reduce_sum(s1[:], e[:].rearrange("p (f c) -> p f c", c=C), axis=X)
            axp = ip.tile([128, fc], F32, tag="axp")
            nc.gpsimd.tensor_mul(axp[:], at[:], xt[:])
            ax = ip.tile([128, f], F32, tag="ax")
            nc.vector.reduce_sum(ax[:], axp[:].rearrange("p (f c) -> p f c", c=C), axis=X)
            lse = ip.tile([128, f], F32, tag="lse")
            nc.scalar.activation(lse[:], s1[:], Act.Ln)
            mr = ip.tile([128, f], F32, tag="mr")
            nc.vector.tensor_sub(mr[:], lse[:], ax[:])

            # --- CE from probs: cp = <y, log clip(x / sum x)> ---
            s2 = ip.tile([128, f], F32, tag="s2")
            nc.vector.reduce_sum(s2[:], x3, axis=X)
            r2 = ip.tile([128, f], F32, tag="r2")
            nc.vector.reciprocal(r2[:], s2[:])
            p = ip.tile([128, fc], F32, tag="p")
            r2b = r2[:, :, None].broadcast_to([128, f, C])
            nc.vector.tensor_mul(p[:].rearrange("p (f c) -> p f c", c=C), x3, r2b)
            nc.vector.tensor_scalar(p[:], p[:], EPS, 1.0 - EPS, Op.max, Op.min)
            lp = ip.tile([128, fc], F32, tag="lp")
            nc.scalar.activation(lp[:], p[:], Act.Ln)
            alpp = ip.tile([128, fc], F32, tag="alpp")
            nc.gpsimd.tensor_mul(alpp[:], at[:], lp[:])
            nc.vector.reduce_sum(
                cp[:, bf], alpp[:].rearrange("p (f c) -> p f c", c=C), axis=X
            )

            # --- ranking mask: valid negatives only ---
            nv = ip.tile([128, f], mybir.dt.int32, tag="nv")
            nc.vector.tensor_sub(nv[:], mskt[:], posm[:, bf])
            nc.vector.copy_predicated(mrm[:, bf], nv[:], mr[:])

        # ---- cross-partition totals ----
        kps = pp.tile([128, n_img], F32)
        nc.tensor.matmul(kps[:], ones128[:], pc_img[:], start=True, stop=True)
        kimg = per.tile([128, n_img], F32)
        nc.vector.tensor_scalar(kimg[:], kps[:], NEG_POS_RATIO, None, Op.mult)

        # fixed bisection bounds (no cross-partition min/max primitive here);
        # |mr| is bounded by ~|logsumexp| + C*max|y*x| << 1e4 for sane inputs.
        hi_t = per.tile([128, n_img], F32)
        nc.vector.memset(hi_t[:], BISECT_BOUND)
        lo_t = per.tile([128, n_img], F32)
        nc.vector.memset(lo_t[:], -BISECT_BOUND)

        # ---- bisection for per-image rank-k threshold ----
        mr3 = mrm[:].rearrange("p (b f) -> p b f", b=n_img)
        mid = per.tile([128, n_img], F32)
        cmp_t = per.tile([128, n_img * f], F32)
        cmp3 = cmp_t[:].rearrange("p (b f) -> p b f", b=n_img)
        cnt = per.tile([128, n_img], F32)
        junk = per.tile([128, 1], F32)
        ge = per.tile([128, n_img], mybir.dt.int32)
        lt = per.tile([128, n_img], mybir.dt.int32)
        for _t in range(T_BISECT):
            nc.vector.tensor_add(mid[:], lo_t[:], hi_t[:])
            nc.vector.tensor_scalar_mul(mid[:], mid[:], 0.5)
            midb = mid[:, :, None].broadcast_to([128, n_img, f])
            nc.vector.tensor_tensor(cmp3, mr3, midb, op=Op.is_ge)
            nc.vector.reduce_sum(cnt[:], cmp3, axis=X)
            cps = pp.tile([128, n_img], F32, tag="cps")
            nc.tensor.matmul(cps[:], ones128[:], cnt[:], start=True, stop=True)
            nc.vector.tensor_tensor(ge[:], cps[:], kimg[:], op=Op.is_ge)
            nc.vector.tensor_tensor(lt[:], cps[:], kimg[:], op=Op.is_lt)
            nc.vector.copy_predicated(lo_t[:], ge[:], mid[:])
            nc.vector.copy_predicated(hi_t[:], lt[:], mid[:])

        # ---- final masked sums ----
        lob = lo_t[:, :, None].broadcast_to([128, n_img, f])
        nc.vector.tensor_tensor(cmp3, mr3, lob, op=Op.is_ge)  # selected negs
        nc.vector.tensor_add(cmp_t[:], cmp_t[:], posm[:])     # | positives
        sc = per.tile([128, n_img * f], F32)
        csum = per.tile([128, 1], F32)
        nc.vector.tensor_mul(sc[:], cp[:], cmp_t[:])
        nc.vector.reduce_sum(csum[:], sc[:], axis=X)
        hsum = per.tile([128, 1], F32)
        nc.vector.reduce_sum(hsum[:], hpp[:], axis=X)
        ptot = per.tile([128, 1], F32)
        nc.vector.reduce_sum(ptot[:], pc_img[:], axis=X)

        pk = per.tile([128, 4], F32)
        nc.vector.memset(pk[:], 0.0)
        nc.vector.tensor_copy(pk[:, 0:1], hsum[:])
        nc.vector.tensor_copy(pk[:, 1:2], csum[:])
        nc.vector.tensor_copy(pk[:, 2:3], ptot[:])
        pkr = pp.tile([128, 4], F32)
        nc.tensor.matmul(pkr[:], ones128[:], pk[:], start=True, stop=True)
        outt = per.tile([1, 4], F32)
        i_cp = nc.vector.tensor_copy(outt[:], pkr[0:1, :])
        i_dma = nc.sync.dma_start(out[:], outt[:])

        # funnel waits so the tail drain needs few sem waits
        n1 = nc.sync.nop()
        add_dep_helper(n1.ins, i_cp.ins, sync=True, reason="funnel-dve")
        n2 = nc.sync.nop()
        add_dep_helper(n2.ins, i_dma.ins, sync=True, reason="funnel-dma")

    return nc


def build_bass(n_img=NIMG, f=F):
    np_tok = 128 * f
    nc = bass.Bass()
    pl = nc.dram_tensor("pl", [n_img, np_tok, C], F32, kind="ExternalInput")
    al = nc.dram_tensor("al", [n_img, np_tok, C], F32, kind="ExternalInput")
    pd = nc.dram_tensor("pd", [n_img, np_tok, 4], F32, kind="ExternalInput")
    ad = nc.dram_tensor("ad", [n_img, np_tok, 4], F32, kind="ExternalInput")
    msk = nc.dram_tensor("msk", [128, f], F32, kind="ExternalInput")
    out = nc.dram_tensor("out", [1, 4], F32, kind="ExternalOutput")
    emit_program(nc, pl, al, pd, ad, msk, out, n_img, f)
    return _patch_wait_splitting(nc)


def _pad_tokens(x, npad, fill):
    """[B, N, D] -> [B, npad, D] padded with `fill` along tokens."""
    b, n, d = x.shape
    if n == npad:
        return np.ascontiguousarray(x, dtype=np.float32)
    out = np.full((b, npad, d), fill, dtype=np.float32)
    out[:, :n, :] = x
    return out


def kernel(actual_bbox_deltas, actual_labels, pred_bbox_deltas, pred_labels):
    global LAST_RESULTS
    ab = np.asarray(actual_bbox_deltas, dtype=np.float32)
    al_ = np.asarray(actual_labels, dtype=np.float32)
    pb = np.asarray(pred_bbox_deltas, dtype=np.float32)
    pl_ = np.asarray(pred_labels, dtype=np.float32)
    assert pl_.shape == (B, N, C), pl_.shape

    # Pad tokens to 128*F. Padded pred_labels rows are all-ones (safe for
    # exp/log); padded labels/deltas are zero, and the msk input excludes
    # padded tokens from negative mining.
    plp = _pad_tokens(pl_, NPAD, 1.0)
    alp = _pad_tokens(al_, NPAD, 0.0)
    pbp = _pad_tokens(pb, NPAD, 0.0)
    abp = _pad_tokens(ab, NPAD, 0.0)

    tok = np.arange(NPAD).reshape(128, F)
    msk = (tok < N).astype(np.float32)

    nc = build_bass()
    in_maps = []
    for c in range(NCORES):
        sl = slice(c * NIMG, (c + 1) * NIMG)
        in_maps.append(
            {
                "pl": np.ascontiguousarray(plp[sl]),
                "al": np.ascontiguousarray(alp[sl]),
                "pd": np.ascontiguousarray(pbp[sl]),
                "ad": np.ascontiguousarray(abp[sl]),
                "msk": msk,
            }
        )

    trace = bool(int(os.environ.get("KERNEL_TRACE", "0")))
    res = run_bass_kernel_spmd(
        nc, in_maps, core_ids=list(range(NCORES)), trace=trace
    )
    LAST_RESULTS = res

    hub_sum = 0.0
    cesel_sum = 0.0
    pos_total = 0.0
    for r in res.results:
        o = r["out"].reshape(-1)
        hub_sum += float(o[0])
        cesel_sum += float(o[1])
        pos_total += float(o[2])

    total_pos = max(pos_total, 1.0)
    loc_loss = np.float32(0.25 * hub_sum / total_pos)
    conf_loss = np.float32(-cesel_sum / total_pos)
    return loc_loss, conf_loss
